# revision 23
# baseline (speedup 1.0000x reference)
"""Trainium2 Bass kernel: CRATEmbedding GNN message passing, 8-core SPMD.

Single-launch design. Nodes (and their out-edges) are sharded across 8 cores.
Per layer, each core computes its local sdst = 0.5*(xi @ W_dst + b) feature-
major, the shards are exchanged with an on-device AllGather, and the per-edge
sdst[edge_dst] gather runs on GPSIMD via indirect_copy: partition group r
(16 partitions) holds the fp16 feature-major sdst table of core r's node
shard, and every edge tile is slotted so its position mod 8 equals its dst
owner core. Edge tiles are (src-supergroup-of-128 x dst-core) cells, 5 tiles
per cell, so the segment sum is one-hot matmuls accumulated over each
supergroup's 40 tiles in PSUM. The radial basis and cosine switch are
computed on device from distances (the 0.5 cutoff factor is folded into
W_dst). Species embedding is an on-device one-hot matmul; layer norm + silu
run feature-major with matmul-based partition reductions/broadcasts. All
heavy tensors are fp16 (tolerance 2e-2; fp16 adds ~0.1%), PSUM accumulation
is f32. Output returns as fp16 and is cast to f32 on host.
"""
import sys

for _p in ("/opt/trn_rl_repo",):
    if _p not in sys.path:
        sys.path.insert(0, _p)

import math
import numpy as np
from contextlib import ExitStack

import concourse.bass as bass
import concourse.mybir as mybir
import concourse.tile as tile
from concourse.masks import make_identity

F32 = mybir.dt.float32
F16 = mybir.dt.float16
U8 = mybir.dt.uint8
U16 = mybir.dt.uint16
AF = mybir.ActivationFunctionType
ALU = mybir.AluOpType

# ---- problem constants ----
N_NODES = 50000
N_EDGES = 1600000
DIM = 256
DSRC = 64
DDST = 16
NB = 8
NLAYERS = 2
NSPECIES = 64
CUTOFF = 5.0
NCORES = 8
P = 128
SG = 128          # src supergroup width == one-hot width
CH = 128          # tiles per chunk

_BUILD_CACHE = {}
LAST_EXEC_NS = None
LAST_RESULTS = None
TRACE = False
DEBUG_TAPS = False
WARMUP = 1


def _ceil_to(x, m):
    return (x + m - 1) // m * m


# ----------------------------------------------------------------------------
# Host-side prep: shard + slot edges into (src-supergroup x dst-core) cells.
# ----------------------------------------------------------------------------
def _prep(edge_src, edge_dst, distances):
    nloc = N_NODES // NCORES            # 6250
    nlp = _ceil_to(nloc, P)             # 6272
    ntn = nlp // P                      # 49 node tiles per core
    ngrp = nlp // SG                    # 49 src supergroups per core

    src = edge_src.astype(np.int64)
    dst = edge_dst.astype(np.int64)
    core = src // nloc
    lsrc = src - core * nloc
    G = lsrc // SG
    srel_all = (lsrc % SG).astype(np.uint8)
    r = dst // nloc                     # dst owner core == gather group
    dloc_all = (dst - r * nloc).astype(np.uint16)

    cell = (core * ngrp + G) * NCORES + r
    ncell = NCORES * ngrp * NCORES
    cnt = np.bincount(cell, minlength=ncell)
    tgc = int(max(1, math.ceil(cnt.max() / P)))   # tiles per cell (uniform)
    tpg = tgc * NCORES                  # tiles per supergroup (40 when tgc=5)
    ntile = ngrp * tpg                  # real tiles per core
    nchunk = math.ceil(ntile / CH)
    ntile_pad = nchunk * CH
    ep = ntile_pad * P

    order = np.argsort(cell, kind="stable")
    cell_s = cell[order]
    starts = np.concatenate([[0], np.cumsum(cnt)[:-1]])
    rank = np.arange(len(src)) - starts[cell_s]
    core_s = cell_s // (ngrp * NCORES)
    G_s = (cell_s // NCORES) % ngrp
    r_s = cell_s % NCORES
    t_in_core = G_s * tpg + (rank // P) * NCORES + r_s
    slot = t_in_core * P + rank % P

    dist = np.full((NCORES, ep), CUTOFF, np.float32)   # pad d=5 -> rbsw=0
    srel = np.zeros((NCORES, ep), np.uint8)
    dloc = np.zeros((NCORES, ep), np.uint16)
    for c in range(NCORES):
        m = core_s == c
        s = slot[m]
        eids = order[m]
        dist[c, s] = distances[eids]
        srel[c, s] = srel_all[eids]
        dloc[c, s] = dloc_all[eids]

    # device layouts
    # dist/srel: slot=(c0*CH+k)*P+e -> [c0, e, k]
    dist_dma = np.ascontiguousarray(
        dist.reshape(NCORES, nchunk, CH, P).transpose(0, 1, 3, 2)).astype(np.float16)
    srel_dma = np.ascontiguousarray(
        srel.reshape(NCORES, nchunk, CH, P).transpose(0, 1, 3, 2))
    # idx: wrapped per 16-partition group: [c0, 16*rr + e%16, kk*8 + e//16]
    A = dloc.reshape(NCORES, nchunk, CH // 8, 8, 8, 16)  # [c, c0, kk, rr, ehi, elo]
    idx_dma = np.ascontiguousarray(
        A.transpose(0, 1, 3, 5, 2, 4).reshape(NCORES, nchunk, P, CH))

    cfg = dict(nloc=nloc, nlp=nlp, ntn=ntn, ngrp=ngrp, tgc=tgc, tpg=tpg,
               ntile=ntile, nchunk=nchunk, ep=ep)
    arrs = dict(dist_dma=dist_dma, srel_dma=srel_dma, idx_dma=idx_dma)
    return cfg, arrs


def _prep_weights(species, W_species, W_src, b_src, W_dst, b_dst, W_mix, b_mix,
                  cfg):
    nloc, nlp = cfg["nloc"], cfg["nlp"]
    w = {}
    w["Wspec"] = np.ascontiguousarray(W_species.astype(np.float16))  # [64,256]
    w["Wsrc"] = np.ascontiguousarray(
        W_src.astype(np.float16).reshape(NLAYERS, 2, 128, DSRC))
    # fold the 0.5 of the cosine switch into W_dst/b_dst
    w["Wdst"] = np.ascontiguousarray(
        (0.5 * W_dst).astype(np.float16).reshape(NLAYERS, 2, 128, DDST))
    wm = W_mix.astype(np.float16)  # [L, 448, 256]
    w["Wmix01"] = np.ascontiguousarray(wm[:, :256].reshape(NLAYERS, 2, 128, DIM))
    w["Wmix2"] = np.ascontiguousarray(wm[:, 256:256 + DSRC])       # [L,64,256]
    w["Wmix3"] = np.ascontiguousarray(wm[:, 256 + DSRC:])          # [L,128,256]
    w["bsrc"] = np.ascontiguousarray(
        b_src.astype(np.float32).reshape(NLAYERS, DSRC, 1))
    w["bdstT"] = np.ascontiguousarray(
        (0.5 * b_dst).astype(np.float32).reshape(NLAYERS, DDST, 1))
    w["bmix"] = np.ascontiguousarray(
        b_mix.astype(np.float32).reshape(NLAYERS, 2, 128, 1))
    w["iota128"] = np.ascontiguousarray(
        np.tile(np.arange(P, dtype=np.float16), (P, 1)))           # [P,128]
    w["iotaP64"] = np.ascontiguousarray(
        np.arange(NSPECIES, dtype=np.float32).reshape(NSPECIES, 1))
    centers = np.linspace(0.0, CUTOFF, NB).astype(np.float64)
    sigma = CUTOFF / NB
    w["cb"] = np.ascontiguousarray(
        (-centers / sigma).astype(np.float32).reshape(NB, 1))      # [8,1]
    # species rows per core, [1, nlp] u8
    sp = species.astype(np.uint8)
    spad = np.zeros((NCORES, 1, nlp), np.uint8)
    for c in range(NCORES):
        spad[c, 0, :nloc] = sp[c * nloc:(c + 1) * nloc]
    w["spec_rows"] = spad
    return w


# ----------------------------------------------------------------------------
# Device program
# ----------------------------------------------------------------------------
def build(cfg):
    nlp = cfg["nlp"]
    ntn = cfg["ntn"]
    ngrp = cfg["ngrp"]
    tpg = cfg["tpg"]
    ntile = cfg["ntile"]
    nchunk = cfg["nchunk"]
    sigma = CUTOFF / NB
    nblk = [(i * 512, min(512, nlp - i * 512)) for i in range(math.ceil(nlp / 512))]

    nc = bass.Bass()
    dp = nc.declare_dram_parameter
    d_spec = dp("spec", [1, nlp], U8, isOutput=False)
    d_dist = dp("dist", [nchunk, P, CH], F16, isOutput=False)
    d_srel = dp("srel", [nchunk, P, CH], U8, isOutput=False)
    d_idx = dp("idxw", [nchunk, P, CH], U16, isOutput=False)
    d_wspec = dp("Wspec", [NSPECIES, DIM], F16, isOutput=False)
    d_wsrc = dp("Wsrc", [NLAYERS, 2, 128, DSRC], F16, isOutput=False)
    d_wdst = dp("Wdst", [NLAYERS, 2, 128, DDST], F16, isOutput=False)
    d_wmix01 = dp("Wmix01", [NLAYERS, 2, 128, DIM], F16, isOutput=False)
    d_wmix2 = dp("Wmix2", [NLAYERS, DSRC, DIM], F16, isOutput=False)
    d_wmix3 = dp("Wmix3", [NLAYERS, P, DIM], F16, isOutput=False)
    d_bsrc = dp("bsrc", [NLAYERS, DSRC, 1], F32, isOutput=False)
    d_bdstT = dp("bdstT", [NLAYERS, DDST, 1], F32, isOutput=False)
    d_bmix = dp("bmix", [NLAYERS, 2, 128, 1], F32, isOutput=False)
    d_iota = dp("iota128", [P, P], F16, isOutput=False)
    d_iotaP = dp("iotaP64", [NSPECIES, 1], F32, isOutput=False)
    d_out = dp("out_xi", [nlp, DIM], F16, isOutput=True)
    taps = {}
    if DEBUG_TAPS:
        taps["xi0T"] = dp("tap_xi0T", [2, P, nlp], F16, isOutput=True)
        taps["sdstT0"] = dp("tap_sdstT0", [DDST, nlp], F16, isOutput=True)
        taps["table0"] = dp("tap_table0", [P, nlp], F16, isOutput=True)
        taps["gath0"] = dp("tap_gath0", [P, 8 * P], F16, isOutput=True)
        taps["rbsw0"] = dp("tap_rbsw0", [P, CH * NB], F16, isOutput=True)
        taps["mi0"] = dp("tap_mi0", [P, nlp], F16, isOutput=True)
        taps["si0"] = dp("tap_si0", [DSRC, nlp], F16, isOutput=True)

    with tile.TileContext(nc) as tc, ExitStack() as ctx:
        cpool = ctx.enter_context(tc.tile_pool(name="const", bufs=1))
        big = ctx.enter_context(tc.tile_pool(name="big", bufs=1))
        spool = ctx.enter_context(tc.tile_pool(name="stat", bufs=2))
        hpool = ctx.enter_context(tc.tile_pool(name="hact", bufs=2))
        epool = ctx.enter_context(tc.tile_pool(name="edge", bufs=1))
        opool = ctx.enter_context(tc.tile_pool(name="ohp", bufs=1))
        mpool = ctx.enter_context(tc.tile_pool(name="mij", bufs=1))
        dram = ctx.enter_context(tc.tile_pool(name="dramcc", bufs=2, space="DRAM"))
        pph = ctx.enter_context(tc.tile_pool(name="ph", bufs=2, space="PSUM"))
        pps = ctx.enter_context(tc.tile_pool(name="ps", bufs=1, space="PSUM"))
        ppb = ctx.enter_context(tc.tile_pool(name="pb", bufs=1, space="PSUM"))
        ppt = ctx.enter_context(tc.tile_pool(name="pt", bufs=2, space="PSUM"))
        ppmi = ctx.enter_context(tc.tile_pool(name="pmi", bufs=1, space="PSUM"))

        # ---- constants ----
        ident16 = cpool.tile([P, P], F16, tag="ident16")
        make_identity(nc, ident16[:])
        iota128 = cpool.tile([P, P], F16, tag="iota128")
        nc.sync.dma_start(out=iota128[:], in_=d_iota[:, :])
        iotaP = cpool.tile([NSPECIES, 1], F32, tag="iotaP")
        nc.sync.dma_start(out=iotaP[:], in_=d_iotaP[:, :])
        eps1 = cpool.tile([P, 1], F32, tag="eps1")
        nc.gpsimd.memset(eps1[:], 1e-6)
        halfpi = cpool.tile([P, 1], F32, tag="halfpi")
        nc.gpsimd.memset(halfpi[:], -math.pi / 2)
        centers_np = np.linspace(0.0, CUTOFF, NB)
        cvec = cpool.tile([P, NB], F32, tag="cvec")
        for b in range(NB):
            nc.gpsimd.memset(cvec[:, b:b + 1], float(centers_np[b]) / sigma)
        ones128 = cpool.tile([P, 1], F16, tag="ones128")
        nc.gpsimd.memset(ones128[:], 1.0)
        ones1x64 = cpool.tile([1, DSRC], F16, tag="ones1x64")
        nc.gpsimd.memset(ones1x64[:], 1.0)
        ones1x128 = cpool.tile([1, P], F16, tag="ones1x128")
        nc.gpsimd.memset(ones1x128[:], 1.0)

        def load_const(src_ap, shape, dt, tag):
            t = cpool.tile(shape, dt, tag=tag, name=tag)
            nc.sync.dma_start(out=t[:], in_=src_ap)
            return t

        wspec = load_const(d_wspec[:, :], [NSPECIES, DIM], F16, "wspec")
        wsrc = [[load_const(d_wsrc[l, c], [128, DSRC], F16, f"wsrc{l}{c}")
                 for c in range(2)] for l in range(NLAYERS)]
        wdst = [[load_const(d_wdst[l, c], [128, DDST], F16, f"wdst{l}{c}")
                 for c in range(2)] for l in range(NLAYERS)]
        wmix01 = [[load_const(d_wmix01[l, c], [128, DIM], F16, f"wm01{l}{c}")
                   for c in range(2)] for l in range(NLAYERS)]
        wmix2 = [load_const(d_wmix2[l], [DSRC, DIM], F16, f"wm2{l}")
                 for l in range(NLAYERS)]
        wmix3 = [load_const(d_wmix3[l], [P, DIM], F16, f"wm3{l}")
                 for l in range(NLAYERS)]
        bsrc = [load_const(d_bsrc[l], [DSRC, 1], F32, f"bsrc{l}")
                for l in range(NLAYERS)]
        bdstT = [load_const(d_bdstT[l], [DDST, 1], F32, f"bdstT{l}")
                 for l in range(NLAYERS)]
        bmix = [[load_const(d_bmix[l, c], [128, 1], F32, f"bmix{l}{c}")
                 for c in range(2)] for l in range(NLAYERS)]

        # persistent activations (fp16)
        xiT = [[big.tile([P, nlp], F16, tag=f"xiT{a}{c}", name=f"xiT{a}{c}")
                for c in range(2)] for a in range(2)]                     # ping-pong per layer
        siT = big.tile([DSRC, nlp], F16, tag="siT")
        miT = big.tile([P, nlp], F16, tag="miT")
        sdstT = big.tile([DDST, nlp], F16, tag="sdstT")
        table = big.tile([P, nlp], F16, tag="table")
        spec16 = cpool.tile([1, nlp], F16, tag="spec16")

        # ------------------------------------------------------------------
        # Feature-major layernorm (optionally silu+bias first).
        # ph(c): psum tiles [128, nw] f32 for the two feature halves.
        # Writes fp16 into out_halves[c][:, off:off+nw].
        # ------------------------------------------------------------------
        def ln_block(ph, off, nw, out_halves, act, biases):
            hb = []
            for c in range(2):
                h = hpool.tile([P, 512], F16, tag="hb")
                if biases is None:
                    nc.scalar.activation(h[:, :nw], ph[c][:, :nw], act,
                                         scale=1.0)
                else:
                    nc.scalar.activation(h[:, :nw], ph[c][:, :nw], act,
                                         bias=biases[c][:, 0:1], scale=1.0)
                hb.append(h)
            s1 = pps.tile([1, 512], F32, tag="st")
            for c in range(2):
                nc.tensor.matmul(s1[:, :nw], ones128[:], hb[c][:, :nw],
                                 start=(c == 0), stop=(c == 1))
            mu = spool.tile([1, 512], F32, tag="mu")
            nc.scalar.activation(mu[:, :nw], s1[:, :nw], AF.Identity,
                                 scale=1.0 / DIM)
            sq = hpool.tile([P, 512], F16, tag="sq")
            s2 = pps.tile([1, 512], F32, tag="st")
            for c in range(2):
                nc.vector.tensor_tensor(out=sq[:, :nw], in0=hb[c][:, :nw],
                                        in1=hb[c][:, :nw], op=ALU.mult)
                nc.tensor.matmul(s2[:, :nw], ones128[:], sq[:, :nw],
                                 start=(c == 0), stop=(c == 1))
            ex2 = spool.tile([1, 512], F32, tag="ex2")
            a_ = spool.tile([1, 512], F32, tag="a_")
            b_ = spool.tile([1, 512], F32, tag="b_")
            nc.scalar.activation(ex2[:, :nw], s2[:, :nw], AF.Identity,
                                 scale=1.0 / DIM)
            nc.vector.tensor_tensor(out=a_[:, :nw], in0=mu[:, :nw],
                                    in1=mu[:, :nw], op=ALU.mult)
            nc.vector.tensor_tensor(out=a_[:, :nw], in0=ex2[:, :nw],
                                    in1=a_[:, :nw], op=ALU.subtract)
            nc.scalar.activation(a_[:, :nw], a_[:, :nw], AF.Sqrt,
                                 bias=eps1[0:1, 0:1], scale=1.0)
            nc.vector.reciprocal(a_[:, :nw], a_[:, :nw])
            nc.vector.tensor_tensor(out=b_[:, :nw], in0=mu[:, :nw],
                                    in1=a_[:, :nw], op=ALU.mult)
            a16 = spool.tile([1, 512], F16, tag="a16")
            b16 = spool.tile([1, 512], F16, tag="b16")
            nc.vector.tensor_copy(a16[:, :nw], a_[:, :nw])
            nc.scalar.activation(b16[:, :nw], b_[:, :nw], AF.Identity,
                                 scale=-1.0)
            abc = ppb.tile([P, 512], F32, tag="bc")
            nc.tensor.matmul(abc[:, :nw], ones1x128[:], a16[:, :nw],
                             start=True, stop=True)
            ca = hpool.tile([P, 512], F16, tag="ca")
            nc.vector.tensor_copy(ca[:, :nw], abc[:, :nw])
            bbc = ppb.tile([P, 512], F32, tag="bc")
            nc.tensor.matmul(bbc[:, :nw], ones1x128[:], b16[:, :nw],
                             start=True, stop=True)
            cbb = hpool.tile([P, 512], F16, tag="cbb")
            nc.vector.tensor_copy(cbb[:, :nw], bbc[:, :nw])
            for c in range(2):
                tmp = hpool.tile([P, 512], F16, tag="tmp")
                nc.vector.tensor_tensor(out=tmp[:, :nw], in0=hb[c][:, :nw],
                                        in1=ca[:, :nw], op=ALU.mult)
                nc.vector.tensor_tensor(out=out_halves[c][:, off:off + nw],
                                        in0=tmp[:, :nw], in1=cbb[:, :nw],
                                        op=ALU.add)

        # ------------------------------------------------------------------
        # Phase 0: species embedding -> LN -> xiT[0]
        # ------------------------------------------------------------------
        spec_u8 = cpool.tile([1, nlp], U8, tag="spec_u8")
        nc.sync.dma_start(out=spec_u8[:], in_=d_spec[:, :])
        nc.vector.tensor_copy(spec16[:], spec_u8[:])
        for off, nw in nblk:
            sbc = pps.tile([NSPECIES, 512], F32, tag="pn", name="sbc")
            nc.tensor.matmul(sbc[:, :nw], ones1x64[:], spec16[:, off:off + nw],
                             start=True, stop=True)
            ohT = hpool.tile([NSPECIES, 512], F16, tag="ohT")
            nc.vector.tensor_tensor(
                out=ohT[:, :nw], in0=sbc[:, :nw],
                in1=iotaP[:].to_broadcast([NSPECIES, nw]), op=ALU.is_equal)
            ph = []
            for c in range(2):
                p_ = pph.tile([P, 512], F32, tag="ph")
                nc.tensor.matmul(p_[:, :nw], wspec[:, c * 128:(c + 1) * 128],
                                 ohT[:, :nw], start=True, stop=True)
                ph.append(p_)
            ln_block(ph, off, nw, xiT[0], AF.Identity, None)
        if DEBUG_TAPS:
            for c in range(2):
                nc.sync.dma_start(out=taps["xi0T"][c], in_=xiT[0][c][:])

        # ------------------------------------------------------------------
        # Layers
        # ------------------------------------------------------------------
        for l in range(NLAYERS):
            xin = xiT[l % 2]
            xout = xiT[(l + 1) % 2]
            # ---- sdstT (feature-major, fp16, 0.5-folded) ----
            for off, nw in nblk:
                pn = pps.tile([DSRC, 512], F32, tag="pn", name="pnd")
                psd = pn[0:DDST, :]
                for c in range(2):
                    nc.tensor.matmul(psd[:, :nw], wdst[l][c][:],
                                     xin[c][:, off:off + nw],
                                     start=(c == 0), stop=(c == 1))
                nc.scalar.activation(sdstT[:, off:off + nw], psd[:, :nw],
                                     AF.Identity, bias=bdstT[l][:, 0:1],
                                     scale=1.0)
            if DEBUG_TAPS and l == 0:
                nc.sync.dma_start(out=taps["sdstT0"][:, :], in_=sdstT[:])
            # ---- AllGather sdstT across cores -> table ----
            ag_in = dram.tile([DDST, nlp], F16, tag=f"agin{l}")
            ag_out = dram.tile([P, nlp], F16, tag=f"agout{l}")
            nc.sync.dma_start(out=ag_in[:], in_=sdstT[:])
            nc.gpsimd.collective_compute(
                "AllGather", ALU.bypass,
                replica_groups=[list(range(NCORES))],
                ins=[ag_in[:].opt()], outs=[ag_out[:].opt()])
            nc.sync.dma_start(out=table[:], in_=ag_out[:])
            if DEBUG_TAPS and l == 0:
                nc.sync.dma_start(out=taps["table0"][:, :], in_=table[:])

            # ---- siT ----
            for off, nw in nblk:
                psi = pps.tile([DSRC, 512], F32, tag="pn", name="pni")
                for c in range(2):
                    nc.tensor.matmul(psi[:, :nw], wsrc[l][c][:],
                                     xin[c][:, off:off + nw],
                                     start=(c == 0), stop=(c == 1))
                nc.scalar.activation(siT[:, off:off + nw], psi[:, :nw],
                                     AF.Identity, bias=bsrc[l][:, 0:1],
                                     scale=1.0)
            if DEBUG_TAPS and l == 0:
                nc.sync.dma_start(out=taps["si0"][:, :], in_=siT[:])

            # ---- edge phase ----
            psum_mi = None
            for c0 in range(nchunk):
                dist_sb = epool.tile([P, CH], F16, tag="dist")
                nc.sync.dma_start(out=dist_sb[:], in_=d_dist[c0])
                srel_sb = epool.tile([P, CH], U8, tag="srelu8")
                nc.sync.dma_start(out=srel_sb[:], in_=d_srel[c0])
                idx_sb = epool.tile([P, CH], U16, tag="idxw")
                nc.sync.dma_start(out=idx_sb[:], in_=d_idx[c0])

                srel16 = epool.tile([P, CH], F16, tag="srel16")
                nc.vector.tensor_copy(srel16[:], srel_sb[:])
                # cos(pi*d/5)+1 == 1 - sin(pi*d/5 - pi/2); keeps Sin arg in range
                sw = epool.tile([P, CH], F16, tag="sw")
                nc.scalar.activation(sw[:], dist_sb[:], AF.Sin,
                                     bias=halfpi[:, 0:1],
                                     scale=math.pi / CUTOFF)
                nc.vector.tensor_scalar(out=sw[:], in0=sw[:], scalar1=-1.0,
                                        scalar2=1.0, op0=ALU.mult,
                                        op1=ALU.add)
                dsc = epool.tile([P, CH], F32, tag="dsc")
                nc.scalar.activation(dsc[:], dist_sb[:], AF.Identity,
                                     scale=1.0 / sigma)
                u2 = epool.tile([P, CH * NB], F16, tag="u2")
                u2v = u2[:].rearrange("p (k b) -> p k b", b=NB)
                nc.vector.tensor_tensor(
                    out=u2v, in0=dsc[:].unsqueeze(2).to_broadcast([P, CH, NB]),
                    in1=cvec[:].unsqueeze(1).to_broadcast([P, CH, NB]),
                    op=ALU.subtract)
                nc.vector.tensor_tensor(out=u2[:], in0=u2[:], in1=u2[:],
                                        op=ALU.mult)
                rbsw = epool.tile([P, CH * NB], F16, tag="rbsw")
                nc.scalar.activation(rbsw[:], u2[:], AF.Exp, scale=-1.0)
                rbv = rbsw[:].rearrange("p (k b) -> p k b", b=NB)
                nc.vector.tensor_tensor(
                    out=rbv, in0=rbv,
                    in1=sw[:].unsqueeze(2).to_broadcast([P, CH, NB]),
                    op=ALU.mult)
                oh_all = opool.tile([P, CH * P], F16, tag="ohall")
                nc.vector.tensor_tensor(
                    out=oh_all[:].rearrange("p (k s) -> p k s", s=P),
                    in0=srel16[:].unsqueeze(2).to_broadcast([P, CH, P]),
                    in1=iota128[:].unsqueeze(1).to_broadcast([P, CH, P]),
                    op=ALU.is_equal)
                if DEBUG_TAPS and l == 0 and c0 == 0:
                    nc.sync.dma_start(out=taps["rbsw0"][:, :], in_=rbsw[:])

                gath = epool.tile([P, CH * DDST], F16, tag="gath")
                half = CH * DDST // 2
                nc.gpsimd.indirect_copy(gath[:, :half], table[:],
                                        idx_sb[:, :CH // 2], True)
                nc.gpsimd.indirect_copy(gath[:, half:], table[:],
                                        idx_sb[:, CH // 2:], True)
                if DEBUG_TAPS and l == 0 and c0 == 0:
                    nc.sync.dma_start(out=taps["gath0"][:, :],
                                      in_=gath[:, :8 * P])

                n_real = min(CH, ntile - c0 * CH)
                n_kk = (n_real + 7) // 8
                for kk4 in range((n_kk + 3) // 4):
                    nq = min(4, n_kk - kk4 * 4)
                    pt4 = ppt.tile([P, 4 * P], F16, tag="pt")
                    for q in range(nq):
                        kk = kk4 * 4 + q
                        nc.tensor.transpose(pt4[:, q * P:(q + 1) * P],
                                            gath[:, kk * P:(kk + 1) * P],
                                            ident16[:])
                    sgt4 = mpool.tile([P, 4 * P], F16, tag="sgt")
                    nc.vector.tensor_copy(sgt4[:], pt4[:])
                    mija = mpool.tile([P, 32 * P], F16, tag="mija")
                    nc.vector.tensor_tensor(
                        out=mija[:].rearrange("p (k b j) -> p k b j",
                                              b=NB, j=DDST),
                        in0=rbv[:, kk4 * 32:(kk4 + 1) * 32, :].unsqueeze(3)
                            .to_broadcast([P, 32, NB, DDST]),
                        in1=sgt4[:].rearrange("p (r j) -> p r j", j=DDST)
                            .unsqueeze(2).to_broadcast([P, 32, NB, DDST]),
                        op=ALU.mult)
                    for kl in range(32):
                        k = kk4 * 32 + kl
                        t = c0 * CH + k
                        if t >= ntile:
                            break
                        Gg, i = divmod(t, tpg)
                        if i == 0:
                            psum_mi = ppmi.tile([P, P], F32, tag="pmi")
                        nc.tensor.matmul(psum_mi[:],
                                         mija[:, kl * P:(kl + 1) * P],
                                         oh_all[:, k * P:(k + 1) * P],
                                         start=(i == 0), stop=(i == tpg - 1))
                        if i == tpg - 1:
                            nc.vector.tensor_copy(
                                miT[:, Gg * P:(Gg + 1) * P], psum_mi[:])
            if DEBUG_TAPS and l == 0:
                nc.sync.dma_start(out=taps["mi0"][:, :], in_=miT[:])

            # ---- W_mix + silu + LN -> xout ----
            for off, nw in nblk:
                ph = []
                for ohalf in range(2):
                    p_ = pph.tile([P, 512], F32, tag="ph")
                    mm = nc.tensor.matmul
                    mm(p_[:, :nw], wmix01[l][0][:, ohalf * 128:(ohalf + 1) * 128],
                       xin[0][:, off:off + nw], start=True, stop=False)
                    mm(p_[:, :nw], wmix01[l][1][:, ohalf * 128:(ohalf + 1) * 128],
                       xin[1][:, off:off + nw], start=False, stop=False)
                    mm(p_[:, :nw], wmix2[l][:, ohalf * 128:(ohalf + 1) * 128],
                       siT[:, off:off + nw], start=False, stop=False)
                    mm(p_[:, :nw], wmix3[l][:, ohalf * 128:(ohalf + 1) * 128],
                       miT[:, off:off + nw], start=False, stop=True)
                    ph.append(p_)
                ln_block(ph, off, nw, xout, AF.Silu, bmix[l])

        # ------------------------------------------------------------------
        # Output: transpose to node-major fp16 and store
        # ------------------------------------------------------------------
        xfin = xiT[NLAYERS % 2]
        for kk in range(ntn):
            ostage = hpool.tile([P, DIM], F16, tag="ostage")
            for c in range(2):
                pt = ppt.tile([P, P], F16, tag="pt")
                nc.tensor.transpose(pt[:], xfin[c][:, kk * P:(kk + 1) * P],
                                    ident16[:])
                nc.vector.tensor_copy(ostage[:, c * 128:(c + 1) * 128], pt[:])
            nc.sync.dma_start(out=d_out[kk * P:(kk + 1) * P, :], in_=ostage[:])

    return nc


def _fix_multiwait_bir(bir_bytes):
    """Walrus here only accepts 1 embedded sync wait per compute instruction;
    move extra waits onto standalone EventSemaphore ops (2 waits each)."""
    import json as _json
    d = _json.loads(bir_bytes)
    for f in d["functions"]:
        for b in f["blocks"]:
            out = []
            for inst in b["instructions"]:
                si = inst.get("sync_info")
                waits = (si or {}).get("on_wait") or []
                eng = inst.get("engine")
                if eng and eng != "Unassigned" and len(waits) > 1:
                    for i, w in enumerate(waits[:-1]):
                        out.append({
                            "debug": inst.get("debug", 0), "engine": eng,
                            "ins": [], "outs": [],
                            "name": "%s-wfix%d" % (inst["name"], i),
                            "opcode": "EventSemaphore",
                            "sync_info": {"on_update": [], "on_wait": [w]}})
                    si["on_wait"] = waits[-1:]
                out.append(inst)
            b["instructions"] = out
    return _json.dumps(d).encode()


_HOOK_PATCHED = False


def _patch_compile_hook():
    global _HOOK_PATCHED
    if _HOOK_PATCHED:
        return
    import concourse.bass2jax as b2j
    orig = b2j.compile_bir_kernel

    def wrapper(bir_json, tmpdir, neff_name="file.neff"):
        return orig(_fix_multiwait_bir(bir_json), tmpdir, neff_name=neff_name)

    b2j.compile_bir_kernel = wrapper
    _HOOK_PATCHED = True


# ----------------------------------------------------------------------------
# Entry point
# ----------------------------------------------------------------------------
def kernel(species, edge_src, edge_dst, distances, switch,
           W_species, W_src, b_src, W_dst, b_dst, W_mix, b_mix):
    global LAST_EXEC_NS, LAST_RESULTS
    species = np.asarray(species)
    edge_src = np.asarray(edge_src)
    edge_dst = np.asarray(edge_dst)
    distances = np.asarray(distances, dtype=np.float32)

    cfg, arrs = _prep(edge_src, edge_dst, distances)
    w = _prep_weights(species, np.asarray(W_species), np.asarray(W_src),
                      np.asarray(b_src), np.asarray(W_dst), np.asarray(b_dst),
                      np.asarray(W_mix), np.asarray(b_mix), cfg)

    key = tuple(sorted((k, v) for k, v in cfg.items()))
    if key not in _BUILD_CACHE:
        _BUILD_CACHE[key] = build(cfg)
    nc = _BUILD_CACHE[key]

    in_maps = []
    for c in range(NCORES):
        in_maps.append(dict(
            spec=w["spec_rows"][c],
            dist=arrs["dist_dma"][c],
            srel=arrs["srel_dma"][c],
            idxw=arrs["idx_dma"][c],
            Wspec=w["Wspec"], Wsrc=w["Wsrc"], Wdst=w["Wdst"],
            Wmix01=w["Wmix01"], Wmix2=w["Wmix2"], Wmix3=w["Wmix3"],
            bsrc=w["bsrc"], bdstT=w["bdstT"], bmix=w["bmix"],
            iota128=w["iota128"], iotaP64=w["iotaP64"],
        ))

    _patch_compile_hook()
    from concourse.bass_utils import run_bass_kernel_spmd

    def launch(trace=False):
        return run_bass_kernel_spmd(nc, in_maps, list(range(NCORES)),
                                    trace=trace)

    for _ in range(WARMUP):
        launch()
    import time as _time
    _t0 = _time.monotonic()
    res = launch(trace=TRACE)
    _wall_ns = int((_time.monotonic() - _t0) * 1e9)
    LAST_EXEC_NS = res.exec_time_ns
    if LAST_EXEC_NS is None:
        # no NTFF hook in this container; report single-launch wall time
        # (includes PJRT dispatch + host<->device transfer, so upper bound)
        LAST_EXEC_NS = _wall_ns
    LAST_RESULTS = res.results
    nloc = cfg["nloc"]
    out = np.concatenate([res.results[c]["out_xi"][:nloc]
                          for c in range(NCORES)], axis=0)
    return out.astype(np.float32)
